# revision 1
# baseline (speedup 1.0000x reference)
"""BernConv (K=2) GNN message passing on 8 Trainium2 NeuronCores.

Self-contained kernel: kernel(**inputs) -> np.ndarray [N, 32] float32.

Strategy: renumber nodes so each of the 8 cores owns a contiguous,
edge-balanced shard (dst-sharding). Work in "g-space" (g = d^-1/2 * f):
    g0 = dh*feat;  g_k = g_{k-1} + dh^2 * agg(g_{k-1})   (k = 1, 2)
    out = s1 * dh^-1 * g2 - s2 * dh * agg(g2)
where agg() gathers src rows (dma_gather over 256B-padded rows from the
DRAM g-table, indices precomputed on host) and segment-sums each dst
node's in-edges with one strided DVE reduce per fixed-degree bucket
chunk. Updated shards are exchanged with AllGather between rounds.
All floating-point arithmetic on feat/weight-derived values runs on
device; the host only does index bookkeeping (graph partitioning).
"""
import sys
sys.path.insert(0, "/opt/trn_rl_repo")

import numpy as np
import concourse.bacc as bacc
import concourse.mybir as mybir
import concourse.tile as tile
from concourse import bass_utils

NC = 8
P = 128
SHARD = 6272
NPAD = NC * SHARD
HALF = NPAD // 2
MAX_IDX_PER_GATHER = 8192
D = 32
EL = 64


NC = 8          # cores
P = 128         # partitions
SHARD = 6272    # nodes per core (128-multiple), 8*6272 = 50176 >= 50000
NPAD = NC * SHARD
HALF = NPAD // 2
MAX_IDX_PER_GATHER = 8192


def bucket_of(k):
    # padded degree: multiples of 2, minimum 2
    return max(2, int(2 * np.ceil(k / 2)))


def build_layout(src, dst, n_nodes):
    E = src.shape[0]
    deg = np.bincount(dst, minlength=n_nodes).astype(np.int64)

    # --- split each node's in-edges by src half (under pi ordering, the halves
    # are cores 0-3 and 4-7; but pi isn't known yet -> two-pass: first assign
    # nodes to cores / pi, then split edges by src's half under pi.

    # pass 1: node -> core assignment. Round-robin deal in degree order so
    # each core gets a near-identical degree profile (keeps the SPMD chunk
    # template tight: per-bucket counts differ across cores by O(1)).
    order = np.argsort(-deg, kind="stable")
    node_core = np.empty(n_nodes, dtype=np.int64)
    node_core[order] = np.arange(n_nodes) % NC

    # provisional pi: cores' nodes in any order; needed to define halves.
    # half(node) = pi_row < HALF  <=>  core(node) < 4
    src_half = (node_core[src] >= 4).astype(np.int64)   # 0 = half A, 1 = B

    # per (node): count of in-edges from each half
    kA = np.zeros(n_nodes, dtype=np.int64)
    kB = np.zeros(n_nodes, dtype=np.int64)
    np.add.at(kA, dst[src_half == 0], 1)
    np.add.at(kB, dst[src_half == 1], 1)
    req = np.maximum(np.maximum(kA, kB), 1)

    # --- choose bucket levels by DP to minimize total padded slots,
    # accounting for 128-node chunk quantization of the per-core max count.
    Lmax = int(req.max())
    # per-core histogram of req
    hist = np.zeros((NC, Lmax + 1), dtype=np.int64)
    for c in range(NC):
        mask = node_core == c
        h = np.bincount(req[mask], minlength=Lmax + 1)
        hist[c, :len(h)] = h
    cum = hist.cumsum(axis=1)     # cum[c, l] = #nodes with req <= l in core c

    def interval_cost(a, b):
        # nodes with req in (a, b] padded to degree b; 128-node quantized max
        need = int((cum[:, b] - cum[:, a]).max())
        return 2 * b * 128 * int(np.ceil(need / 128))

    INF = float("inf")
    fcost = [INF] * (Lmax + 1)
    prev = [0] * (Lmax + 1)
    fcost[0] = 0.0
    for b in range(1, Lmax + 1):
        for a in range(0, b):
            cbd = fcost[a] + interval_cost(a, b)
            if cbd < fcost[b]:
                fcost[b] = cbd
                prev[b] = a
    levels = []
    b = Lmax
    while b > 0:
        levels.append(b)
        b = prev[b]
    levels = sorted(levels)
    lev_arr = np.array(levels, dtype=np.int64)
    ghat = lev_arr[np.searchsorted(lev_arr, req)]

    # pass 2: within each core, group nodes by ghat (desc), build chunk
    # template = per-bucket max chunk count across cores.
    core_nodes = [np.where(node_core == c)[0] for c in range(NC)]
    # order nodes within core by (ghat desc, node id) for determinism
    core_nodes = [cn[np.lexsort((cn, -ghat[cn]))] for cn in core_nodes]

    buckets = sorted(set(ghat.tolist()), reverse=True)
    # per core per bucket: node count
    bcount = {b: [0] * NC for b in buckets}
    for c in range(NC):
        gl = ghat[core_nodes[c]]
        for b in buckets:
            bcount[b][c] = int((gl == b).sum())

    # chunk template: for bucket b, m_b = max(1, 32 // b) nodes/partition,
    # slots per half per chunk = 128*m*b <= 8192
    chunks = []  # list of (b, m) ; repeated per chunk
    for b in buckets:
        m_b = max(1, 32 // b)
        assert 128 * m_b * b <= MAX_IDX_PER_GATHER, (b, m_b)
        need = max(bcount[b][c] for c in range(NC))  # max nodes over cores
        nfull = need // (P * m_b)
        for _ in range(nfull):
            chunks.append((b, m_b))
        # tail chunks at m=1 (waste <= 127 nodes per bucket)
        rem = need - nfull * P * m_b
        while rem > 0:
            chunks.append((b, 1))
            rem -= P

    node_slots = P * sum(m for _, m in chunks)   # node capacity per core
    assert node_slots * NC >= NPAD or True

    # --- assign nodes to (chunk, p, m) slots per core; build pi.
    # pi row of node = core*SHARD + within-core index, where within-core
    # index is defined by slot position: iterating chunks in order, node
    # index = chunk_node_offset + m*128 + p.
    # dummy nodes fill leftover slots (they are NOT part of pi; pi only maps
    # real nodes; dummy slots gather the zero row and scales are 0).
    pi = np.full(n_nodes, -1, dtype=np.int64)         # node -> padded row
    # per core: slot table: for each chunk: node_id or -1, shape [m, P]
    slot_nodes = []  # [core][chunk] -> int64 [m, P] (-1 = dummy)
    for c in range(NC):
        nodes_by_bucket = {}
        gl = ghat[core_nodes[c]]
        for b in buckets:
            nodes_by_bucket[b] = core_nodes[c][gl == b]
        used = {b: 0 for b in buckets}
        rowbase = c * SHARD
        rows_assigned = 0
        per_chunk = []
        for (b, m) in chunks:
            tab = np.full((m, P), -1, dtype=np.int64)
            avail = nodes_by_bucket[b]
            u = used[b]
            take = min(m * P, len(avail) - u)
            if take > 0:
                sel = avail[u:u + take]
                used[b] += take
                flat = tab.reshape(-1)
                flat[:take] = sel
                # pi rows: node at (m_i, p) -> rowbase + rows_assigned + m_i*128 + p
                mi = np.arange(take) // P
                pp = np.arange(take) % P
                pi[sel] = rowbase + rows_assigned + mi * P + pp
            rows_assigned += m * P
            per_chunk.append(tab)
        assert all(used[b] == len(nodes_by_bucket[b]) for b in buckets), c
        slot_nodes.append(per_chunk)
        assert rows_assigned == node_slots

    # shard_pad: per-core padded row count. Add one spare 128-row block so
    # the last row of each half is guaranteed unoccupied (zero row).
    shard_pad = node_slots + P
    pi = np.full(n_nodes, -1, dtype=np.int64)
    for c in range(NC):
        rowbase = c * shard_pad
        rows_assigned = 0
        for (b, m), tab in zip(chunks, slot_nodes[c]):
            flat = tab.reshape(-1)
            take_idx = np.where(flat >= 0)[0]
            if len(take_idx):
                sel = flat[take_idx]
                mi = take_idx // P
                pp = take_idx % P
                pi[sel] = rowbase + rows_assigned + mi * P + pp
            rows_assigned += m * P
    n_pad = NC * shard_pad
    assert 4 * shard_pad <= 32768, ("int16 idx range exceeded", shard_pad)

    half_rows = n_pad // 2
    assert half_rows % 2 == 0
    # zero row per half: we need one padded row per half that is guaranteed
    # zero. Use row half_rows-1 and n_pad-1 ONLY if unoccupied; safer:
    # extend each half by one row? Half size must stay equal for slicing.
    # Instead reserve the LAST row of each half: ensure no node mapped there.
    zrowA = half_rows - 1
    zrowB = n_pad - 1
    assert not (pi == zrowA).any() and not (pi == zrowB).any(), \
        "zero rows occupied; adjust shard padding"

    # --- gather index arrays per core ---------------------------------------
    # edge lists per (dst node, half), sorted by dst then arbitrary
    eorder = np.lexsort((src, src_half, dst))   # sort by dst, then half, then src
    s_dst = dst[eorder]
    s_half = src_half[eorder]
    s_src = src[eorder]
    # boundaries per (dst, half)
    # build per-node lists via searchsorted
    node_edge_start = np.searchsorted(s_dst, np.arange(n_nodes))
    node_edge_end = np.searchsorted(s_dst, np.arange(n_nodes), side="right")

    src_pirow = pi[s_src]         # padded row of each edge's src
    assert (src_pirow >= 0).all()

    idxA_cores, idxB_cores = [], []
    scale_layout_cores = []       # per core: int64 [n_slots_m] node id or -1 in (chunk,m,p) layout
    for c in range(NC):
        idxA_parts, idxB_parts = [], []
        lay_nodes = []
        for (b, m), tab in zip(chunks, slot_nodes[c]):
            # idx block for this chunk: [m*b slots] x 128 partitions ->
            # linear idx list t = (m_i*b + g)*128 + p
            cntA = np.full((m, P, b), zrowA, dtype=np.int64)
            cntB = np.full((m, P, b), zrowB - half_rows, dtype=np.int64)
            for mi in range(m):
                for p in range(P):
                    nd = tab[mi, p]
                    if nd < 0:
                        continue
                    st, en = node_edge_start[nd], node_edge_end[nd]
                    rows = src_pirow[st:en]
                    hh = s_half[st:en]
                    ra = rows[hh == 0]
                    rb = rows[hh == 1] - half_rows
                    assert len(ra) <= b and len(rb) <= b, (len(ra), len(rb), b)
                    cntA[mi, p, :len(ra)] = ra
                    cntB[mi, p, :len(rb)] = rb
            # linear order: t = (mi*b + g)*128 + p
            linA = cntA.transpose(0, 2, 1).reshape(-1)   # [m*b*P] in (mi,g,p)
            linB = cntB.transpose(0, 2, 1).reshape(-1)
            idxA_parts.append(linA)
            idxB_parts.append(linB)
            lay_nodes.append(tab.reshape(-1))            # (mi, p)
        idxA_cores.append(np.concatenate(idxA_parts))
        idxB_cores.append(np.concatenate(idxB_parts))
        scale_layout_cores.append(np.concatenate(lay_nodes))

    return dict(
        pi=pi, deg=deg, chunks=chunks, shard_pad=shard_pad, n_pad=n_pad,
        half_rows=half_rows, zrowA=zrowA, zrowB=zrowB,
        idxA=idxA_cores, idxB=idxB_cores,
        scale_layout=scale_layout_cores, node_core=node_core,
    )


def wrap_idx(lin, seg_lens):
    """dma_gather idx wrapping per chunk segment: [16, cnt/16] tiled to 128."""
    out = []
    off = 0
    for cnt in seg_lens:
        seg = lin[off:off + cnt]
        w = seg.reshape(-1, 16).T            # [16, cnt/16]
        out.append(np.tile(w, (8, 1)))       # [128, cnt/16]
        off += cnt
    assert off == len(lin)
    return np.concatenate(out, axis=1)       # [128, total/16]







def make_host_data(feat, weight, src, dst):
    n_nodes = feat.shape[0]
    lay = build_layout(src, dst, n_nodes)
    chunks = lay["chunks"]
    shard_pad = lay["shard_pad"]
    n_pad = lay["n_pad"]
    pi = lay["pi"]
    S = sum(m for _, m in chunks)
    KB = n_pad // P

    deg = lay["deg"].astype(np.float64)
    degc = np.maximum(deg, 1.0).astype(np.float32)
    dh = (degc ** -0.5).astype(np.float32)
    dh2 = (dh * dh).astype(np.float32)
    dhinv = (1.0 / dh).astype(np.float32)

    featp = np.zeros((n_pad, D), np.float32)
    featp[pi] = feat

    dh_row = np.zeros(n_pad, np.float32)
    dh_row[pi] = dh
    dhini = np.ascontiguousarray(dh_row.reshape(KB, P).T)

    coef = np.array([[0.25, 0.5, 0.25, 0.0, 0.5, 0.25]], np.float32)
    w2 = np.ascontiguousarray(weight.reshape(1, 3).astype(np.float32))

    seg_lens = [P * m * b for b, m in chunks]
    per_core = []
    for c in range(NC):
        idxA_w = wrap_idx(lay["idxA"][c].astype(np.int16), seg_lens)
        idxB_w = wrap_idx(lay["idxB"][c].astype(np.int16), seg_lens)
        sl_nodes = lay["scale_layout"][c].reshape(S, P)   # [s, p]
        valid = sl_nodes >= 0
        nd = np.where(valid, sl_nodes, 0)

        def slot_vec(v):
            out = np.where(valid, v[nd], 0.0).astype(np.float32)  # [s, p]
            return np.ascontiguousarray(out.T)                     # [p, s]

        dh2l = slot_vec(dh2)
        dhhl = slot_vec(dh)
        dhil = slot_vec(dhinv)
        fsl = np.zeros((S, P, D), np.float32)
        fsl[valid] = feat[nd[valid]]
        featsl = np.ascontiguousarray(fsl.transpose(1, 0, 2).reshape(P, S * D))

        per_core.append(dict(
            featp=featp, featsl=featsl, w=w2, coef=coef,
            idxA=np.ascontiguousarray(idxA_w), idxB=np.ascontiguousarray(idxB_w),
            dhini=dhini, dh2l=dh2l, dhhl=dhhl, dhil=dhil,
        ))
    return lay, per_core


DEVICE_KEYS = ("featsl", "w", "coef", "dh2l", "dhhl", "dhil")


def device_inputs(per_core):
    maps = []
    for pc in per_core:
        m = {k: pc[k] for k in DEVICE_KEYS}
        m["idxA"] = np.ascontiguousarray(pc["idxA"][:16])
        m["idxB"] = np.ascontiguousarray(pc["idxB"][:16])
        maps.append(m)
    return maps


def assemble_output(lay, outs, n_nodes):
    """outs: list of per-core 'out' arrays [shard_pad, 32]."""
    pi = lay["pi"]
    shard_pad = lay["shard_pad"]
    node_core = lay["node_core"]
    res = np.empty((n_nodes, D), np.float32)
    for c in range(NC):
        nodes = np.where(node_core == c)[0]
        local = pi[nodes] - c * shard_pad
        res[nodes] = outs[c][local]
    return res




F32 = mybir.dt.float32
I16 = mybir.dt.int16
GMAX = 512  # max idx per dma_gather call (walrus 16-bit sem wait limit)


def build_kernel(chunks, shard_pad, n_pad):
    """chunks: list of (ghat, m) — identical template for all cores."""
    half = n_pad // 2
    S = sum(m for _, m in chunks)           # node slot columns per partition
    TA = sum(P * m * b for b, m in chunks)  # gather slots per half

    nc = bacc.Bacc("TRN2", target_bir_lowering=False, num_swdge_queues=4)
    featsl_d = nc.dram_tensor("featsl", [P, S * D], F32, kind="ExternalInput")
    w_d = nc.dram_tensor("w", [1, 3], F32, kind="ExternalInput")
    coef_d = nc.dram_tensor("coef", [1, 6], F32, kind="ExternalInput")
    idxA_d = nc.dram_tensor("idxA", [16, TA // 16], I16, kind="ExternalInput")
    idxB_d = nc.dram_tensor("idxB", [16, TA // 16], I16, kind="ExternalInput")
    dh2l_d = nc.dram_tensor("dh2l", [P, S], F32, kind="ExternalInput")
    dhhl_d = nc.dram_tensor("dhhl", [P, S], F32, kind="ExternalInput")
    dhil_d = nc.dram_tensor("dhil", [P, S], F32, kind="ExternalInput")
    out_d = nc.dram_tensor("out", [shard_pad, D], F32, kind="ExternalOutput")

    with tile.TileContext(nc) as tc:
        with (
            tc.tile_pool(name="dram", bufs=1, space="DRAM") as dramp,
            tc.tile_pool(name="persist", bufs=1) as persist,
            tc.tile_pool(name="slab", bufs=3) as slabp,
        ):
            g_tabs = [dramp.tile([n_pad, EL], F32, name=f"g{k}") for k in range(3)]
            bounce = [dramp.tile([shard_pad, EL], F32, name=f"bnc{k}") for k in range(3)]

            # gather indices: shipped [16, T/16], replicated to 128 partitions
            # by 8 reads of the same DRAM region.
            idxA = persist.tile([P, TA // 16], I16, name="idxA")
            idxB = persist.tile([P, TA // 16], I16, name="idxB")
            for r in range(8):
                nc.sync.dma_start(out=idxA[r * 16:(r + 1) * 16, :], in_=idxA_d[:])
                nc.sync.dma_start(out=idxB[r * 16:(r + 1) * 16, :], in_=idxB_d[:])
            dh2l = persist.tile([P, S], F32, name="dh2l")
            dhhl = persist.tile([P, S], F32, name="dhhl")
            dhil = persist.tile([P, S], F32, name="dhil")
            nc.sync.dma_start(out=dh2l[:], in_=dh2l_d[:])
            nc.sync.dma_start(out=dhhl[:], in_=dhhl_d[:])
            nc.sync.dma_start(out=dhil[:], in_=dhil_d[:])

            # scalars s1, s2 (relu + weighted sums), broadcast to partitions
            wt = persist.tile([1, 3], F32, name="wt")
            coefs = persist.tile([1, 6], F32, name="coefs")
            nc.sync.dma_start(out=wt[:], in_=w_d[:])
            nc.sync.dma_start(out=coefs[:], in_=coef_d[:])
            wr = persist.tile([1, 3], F32, name="wr")
            nc.vector.tensor_scalar(out=wr[:], in0=wt[:], scalar1=0.0,
                                    scalar2=None, op0=mybir.AluOpType.max)
            sprod = persist.tile([1, 6], F32, name="sprod")
            nc.vector.tensor_tensor(out=sprod[:, 0:3], in0=wr[:],
                                    in1=coefs[:, 0:3], op=mybir.AluOpType.mult)
            nc.vector.tensor_tensor(out=sprod[:, 3:6], in0=wr[:],
                                    in1=coefs[:, 3:6], op=mybir.AluOpType.mult)
            svals = persist.tile([1, 2], F32, name="svals")
            nc.vector.tensor_reduce(out=svals[:, 0:1], in_=sprod[:, 0:3],
                                    axis=mybir.AxisListType.X, op=mybir.AluOpType.add)
            nc.vector.tensor_reduce(out=svals[:, 1:2], in_=sprod[:, 3:6],
                                    axis=mybir.AxisListType.X, op=mybir.AluOpType.add)
            sbc = persist.tile([P, 2], F32, name="sbc")
            nc.gpsimd.partition_broadcast(sbc[:], svals[:])

            # zero tile for the spare 128-row block of each bounce
            zt = persist.tile([P, EL], F32, name="zt")
            nc.gpsimd.memset(zt[:], 0.0)

            # g0 shard = dhhl * featsl; AllGather -> g0 table
            featsl = persist.tile([P, S * D], F32, name="featsl")
            nc.sync.dma_start(out=featsl[:], in_=featsl_d[:])
            gold = persist.tile([P, S * D], F32, name="gold1")
            nc.vector.tensor_tensor(
                out=gold[:].rearrange("p (s d) -> p s d", d=D),
                in0=featsl[:].rearrange("p (s d) -> p s d", d=D),
                in1=dhhl[:].to_broadcast([P, S, D]),
                op=mybir.AluOpType.mult)

            def ship_shard(tile_, bnc, gtab):
                bnc_r = bnc[:].rearrange("(s p) e -> p s e", p=P)
                nc.sync.dma_start(
                    out=bnc_r[:, 0:S, 0:D],
                    in_=tile_[:].rearrange("p (s d) -> p s d", d=D))
                nc.sync.dma_start(out=bnc_r[:, S:S + 1, :], in_=zt[:, None, :])
                nc.gpsimd.collective_compute(
                    "AllGather", mybir.AluOpType.bypass,
                    replica_groups=[list(range(8))],
                    ins=[bnc.opt()], outs=[gtab.opt()])

            ship_shard(gold, bounce[0], g_tabs[0])

            aggA = persist.tile([P, S * D], F32, name="aggA")
            aggB = persist.tile([P, S * D], F32, name="aggB")

            gq = [0]
            for k in (1, 2, 3):
                gsrc = g_tabs[k - 1]
                ioff = 0
                soff = 0
                for ci, (b, m) in enumerate(chunks):
                    cnt = P * m * b
                    slA = slabp.tile([P, m * b * EL], F32, tag="slA", name=f"slA{k}_{ci}")
                    slB = slabp.tile([P, m * b * EL], F32, tag="slB", name=f"slB{k}_{ci}")
                    for g0c in range(0, cnt, GMAX):
                        gc = min(GMAX, cnt - g0c)
                        c0, c1 = g0c // P, (g0c + gc) // P
                        nc.gpsimd.dma_gather(
                            out_ap=slA[:, c0 * EL:c1 * EL].rearrange(
                                "p (c e) -> p c e", e=EL),
                            in_ap=gsrc[0:half, :],
                            idxs_ap=idxA[:, ioff + g0c // 16:ioff + (g0c + gc) // 16],
                            num_idxs=gc, num_idxs_reg=gc, elem_size=EL,
                            single_packet=False, queue_num=gq[0] % 4)
                        gq[0] += 1
                        nc.gpsimd.dma_gather(
                            out_ap=slB[:, c0 * EL:c1 * EL].rearrange(
                                "p (c e) -> p c e", e=EL),
                            in_ap=gsrc[half:, :],
                            idxs_ap=idxB[:, ioff + g0c // 16:ioff + (g0c + gc) // 16],
                            num_idxs=gc, num_idxs_reg=gc, elem_size=EL,
                            single_packet=False, queue_num=gq[0] % 4)
                        gq[0] += 1
                    for sl, agg in ((slA, aggA), (slB, aggB)):
                        nc.vector.tensor_reduce(
                            out=agg[:, soff * D:(soff + m) * D].rearrange(
                                "p (m d) -> p m d", d=D),
                            in_=sl[:].rearrange("p (m g e) -> p m g e", m=m, g=b)[
                                :, :, :, 0:D].rearrange("p m g d -> p m d g"),
                            axis=mybir.AxisListType.X, op=mybir.AluOpType.add)
                    ioff += cnt // 16
                    soff += m

                t = persist.tile([P, S * D], F32, name=f"t{k}", tag=f"t{k}")
                nc.vector.tensor_tensor(out=t[:], in0=aggA[:], in1=aggB[:],
                                        op=mybir.AluOpType.add)
                if k < 3:
                    nc.vector.tensor_tensor(
                        out=t[:].rearrange("p (s d) -> p s d", d=D),
                        in0=t[:].rearrange("p (s d) -> p s d", d=D),
                        in1=dh2l[:].to_broadcast([P, S, D]),
                        op=mybir.AluOpType.mult)
                    nc.vector.tensor_tensor(out=t[:], in0=t[:], in1=gold[:],
                                            op=mybir.AluOpType.add)
                    gold = t
                    ship_shard(t, bounce[k], g_tabs[k])
                else:
                    u = persist.tile([P, S * D], F32, name="u")
                    nc.vector.tensor_tensor(
                        out=u[:].rearrange("p (s d) -> p s d", d=D),
                        in0=gold[:].rearrange("p (s d) -> p s d", d=D),
                        in1=dhil[:].to_broadcast([P, S, D]),
                        op=mybir.AluOpType.mult)
                    nc.vector.tensor_scalar(out=u[:], in0=u[:],
                                            scalar1=sbc[:, 0:1], scalar2=None,
                                            op0=mybir.AluOpType.mult)
                    nc.vector.tensor_tensor(
                        out=t[:].rearrange("p (s d) -> p s d", d=D),
                        in0=t[:].rearrange("p (s d) -> p s d", d=D),
                        in1=dhhl[:].to_broadcast([P, S, D]),
                        op=mybir.AluOpType.mult)
                    nc.vector.tensor_scalar(out=t[:], in0=t[:],
                                            scalar1=sbc[:, 1:2], scalar2=None,
                                            op0=mybir.AluOpType.mult)
                    nc.vector.tensor_tensor(out=u[:], in0=u[:], in1=t[:],
                                            op=mybir.AluOpType.subtract)
                    out_r = out_d[:].rearrange("(s p) d -> p s d", p=P)
                    nc.sync.dma_start(
                        out=out_r[:, 0:S, :],
                        in_=u[:].rearrange("p (s d) -> p s d", d=D))
    nc.compile()
    return nc


_CACHE = {}


def kernel(feat, weight, src, dst):
    feat = np.ascontiguousarray(np.asarray(feat, dtype=np.float32))
    weight = np.ascontiguousarray(np.asarray(weight, dtype=np.float32))
    src64 = np.asarray(src).astype(np.int64)
    dst64 = np.asarray(dst).astype(np.int64)
    n_nodes = feat.shape[0]

    lay, per_core = make_host_data(feat, weight, src64, dst64)
    key = (len(lay["chunks"]), lay["shard_pad"], lay["n_pad"],
           tuple(lay["chunks"]))
    if key not in _CACHE:
        _CACHE[key] = build_kernel(lay["chunks"], lay["shard_pad"], lay["n_pad"])
    nc = _CACHE[key]

    in_maps = device_inputs(per_core)
    res = bass_utils.run_bass_kernel_spmd(nc, in_maps, core_ids=list(range(NC)))
    outs = [res.results[c]["out"] for c in range(NC)]
    return assemble_output(lay, outs, n_nodes)

